# revision 1
# baseline (speedup 1.0000x reference)
"""DSS Mamba (bidirectional selective scan) Trainium2 kernel.

Sharding: 8 cores = 2 directions x 2 batch x 2 halves of d_inner.
Each core:
  - computes in_proj (x rows for its whole direction, z rows for its half),
  - x_proj -> (dt, B, C), dt_proj -> softplus -> delta,
  - selective scan over its 256 channels (d on partitions, L on free dim,
    hardware tensor_tensor_scan along the free dim, 16 states sequentially),
  - gate + partial out_proj (its 256 rows of the 1024-row contraction).
Host flips the sequence for the backward direction and sums the 4 partial
out_proj contributions per batch element.

Engine assignment (cost-model ns for [128,512] ops):
  DVE : all scans (594, only engine that can scan) + all dbu TTs (327,
        same-engine adjacency with the scan is critical) + gate
  Pool: 10/16 states' h*C TTs (1110, gpsimd sw-mult) + hlast copies
  ACT : silu + softplus(exp,ln) + all dA exps (611)
        (exp+ln+copy forced into ONE table via _patch_act_tables;
        silu is the only other table; z-silus emitted after the scan
        stage so they never split the softplus->dA exp run)
  PE  : all matmuls bf16 (1cyc/row): projections + accumulation matmuls
        (16 t_n + diag(D)@u) into per-(m,q) y PSUM tiles.
Precision: bf16 matmul operands/scan streams/output; delta+da fp16;
softplus in fp32.  Measured end-to-end rel err ~7e-3 (budget 2e-2).
Cost-model span 156.6us: fill ~13 + DVE-dense ~134 + tail ~5.
dbu and h*C ops are m-pair-fused ([128,2,LC] tiles; shared B/C row
enters via a free-dim zero-stride broadcast, HW-verified).
"""

import numpy as np
from contextlib import ExitStack

import concourse.bacc as bacc
import concourse.tile as tile
from concourse import mybir
from concourse.bass_utils import run_bass_kernel_spmd

F32 = mybir.dt.float32
F16 = mybir.dt.float16
BF16 = mybir.dt.bfloat16
AF = mybir.ActivationFunctionType
OP = mybir.AluOpType

D_MODEL = 256
D_INNER = 512
N_STATE = 16
DT_RANK = 16
import os
L = 2048
FC = 512           # matmul free-dim chunk
# chunk schedule: small first chunk for fast pipeline fill, small last for tail
CHUNKS = [int(x) for x in os.environ.get("K_CHUNKS", "512,512,512,512").split(",")]
assert sum(CHUNKS) == L
LCMAX = max(CHUNKS)
WPACK_COLS = 1024 + 512 + 256 + 256 + 256 + 512 + 128 + 68

import ast as _ast
POOL_DBU = set(_ast.literal_eval(os.environ.get("K_POOL_DBU", "()")))
POOL_T = set(_ast.literal_eval(os.environ.get("K_POOL_T", "(1,2,3,4,6,7,8,9,11,12,14)")))
GATE_POOL = os.environ.get("K_GATE_POOL", "0") == "1"
MID_N = int(os.environ.get("K_MIDN", "99"))

_CACHE = {}


def _patch_act_tables():
    """Empty the exp-only / ln-only act function sets so the table-load pass
    must pick natural_log_exp_and_others (covers exp+ln+copy in ONE table).
    Set positions (= act_func_set_ids) are preserved."""
    import concourse.bacc as _b
    if getattr(_b, "_act_tables_patched", False):
        return
    _orig = _b.get_activation_tables

    def patched(arch):
        t = _orig(arch)
        out = {}
        for name, s in t.items():
            if name in ("exp_and_others", "natural_log", "exp_and_friends"):
                out[name] = set()
            else:
                out[name] = s
        return out

    _b.get_activation_tables = patched
    _b._act_tables_patched = True


def _build():
    if "nc" in _CACHE:
        return _CACHE["nc"]
    _patch_act_tables()

    nc = bacc.Bacc("TRN2", target_bir_lowering=False, debug=False)

    def din(name, shape, dtype=BF16):
        return nc.dram_tensor(name, shape, dtype, kind="ExternalInput").ap()

    hsT = din("hsT", [2, 128, L])
    # bf16 weights packed into two tensors -> two DMAs at startup
    # (A holds in_proj x-weights, needed first)
    wpackA = din("wpackA", [128, 1024])
    wpackB = din("wpackB", [128, WPACK_COLS - 1024])
    out_ap = nc.dram_tensor("out", [2, 128, L], BF16, kind="ExternalOutput").ap()
    # B/C rows staged to DRAM (bf16), interleaved as (B_n, C_n) row pairs so
    # one broadcast DMA fetches both operands for a state
    bc_dram = nc.dram_tensor("bc_scratch", [1, 32, L], BF16).ap()

    with tile.TileContext(nc) as tc, ExitStack() as ctx:
        const = ctx.enter_context(tc.tile_pool(name="const", bufs=1))
        big = ctx.enter_context(tc.tile_pool(name="big", bufs=2))
        work = ctx.enter_context(tc.tile_pool(name="work", bufs=2))
        psum = ctx.enter_context(tc.tile_pool(name="psum", bufs=3, space="PSUM"))
        psumy = ctx.enter_context(tc.tile_pool(name="psumy", bufs=1, space="PSUM"))

        # ---- load weights ----
        def load_const(ap, shape, tag, dtype=BF16):
            t = const.tile(shape, dtype, tag=tag, name=tag)
            nc.sync.dma_start(out=t[:], in_=ap)
            return t

        # hs chunk-0 slices first: they gate the first in_proj matmuls
        early_hsk = []
        for k in range(2):
            t = const.tile([128, FC], BF16, tag=f"ehsk{k}", name=f"ehsk{k}")
            nc.sync.dma_start(out=t[:], in_=hsT[k][:, 0:FC])
            early_hsk.append(t)
        wpa = load_const(wpackA, [128, 1024], "wpackA")

        # allocate wpackB's tile now; its DMA is issued after the warmup so it
        # doesn't delay the first hs-chunk transfers
        wpb = const.tile([128, WPACK_COLS - 1024], BF16, tag="wpackB", name="wpackB")

        def wslice(off, cols, rows=128):
            return wpb[0:rows, off - 1024:off - 1024 + cols]

        o = 0
        w_in_x_sb = [wpa[:, k * 512:(k + 1) * 512] for k in range(2)]; o += 1024
        w_in_z_sb = [wslice(o + k * 256, 256) for k in range(2)]; o += 512
        w_x_sb = [wslice(o + k * 64, 64) for k in range(4)]; o += 256
        w_dt_sb = wslice(o, 256, rows=16); o += 256
        d_diag_sb = [wslice(o + k * 128, 128) for k in range(2)]; o += 256
        w_out_sb = [wslice(o + k * 256, 256) for k in range(2)]; o += 512
        ident_sb = wslice(o, 128); o += 128
        # fp32 bias/scale block rides in wpackB as raw bytes (bitcast view)
        fp = wslice(o, 68).bitcast(F32); o += 68
        assert o == WPACK_COLS
        bdt_sb = [fp[:, m:m + 1] for m in range(2)]
        a_sc_sb = [fp[:, 2 + m * 16: 2 + (m + 1) * 16] for m in range(2)]

        # PE p-state warmup: matmuls on the first-arriving data so the PE is
        # hot before the first in_proj matmul.
        for _w in range(3):
            wps = psum.tile([34, 256], F32, tag="warm", name="warm", bufs=1)
            nc.tensor.matmul(wps[:], lhsT=early_hsk[0][:, 0:34], rhs=early_hsk[0][:, 0:256],
                             start=True, stop=True, skip_group_check=True)

        # wpackB (xdbl/dt/out weights) loads after the first hs chunks
        nc.sync.dma_start(out=wpb[:], in_=wpackB)

        hlast = [[const.tile([128, 1], F32, tag=f"hl{m}_{n}", name=f"hl{m}_{n}")
                  for n in range(N_STATE)] for m in range(2)]

        def prestage(cid, base, size):
            """Projections for chunk cid ([base, base+size)): (u, z, delta, du)."""
            nf = size // FC
            u = [big.tile([128, LCMAX], BF16, tag=f"u{m}", name=f"u{m}") if m < 2
                 else work.tile([128, LCMAX], BF16, tag=f"u{m}", name=f"u{m}", bufs=2)
                 for m in range(4)]
            z16 = [big.tile([128, LCMAX], BF16, tag=f"z{m}", name=f"z{m}") for m in range(2)]
            xd16 = big.tile([16, LCMAX], BF16, tag="xd16", name="xd16")
            bc16 = big.tile([32, LCMAX], BF16, tag="bc16", name="bc16")
            zjobs = []
            for fc in range(nf):
                fs = slice(fc * FC, (fc + 1) * FC)       # within-chunk
                gs = slice(base + fc * FC, base + (fc + 1) * FC)  # global
                if cid == 0 and fc == 0:
                    hsk = early_hsk
                else:
                    hsk = []
                    for k in range(2):
                        t = work.tile([128, FC], BF16, tag=f"hsk{k}", name=f"hsk{k}", bufs=3)
                        nc.sync.dma_start(out=t[:], in_=hsT[k][:, gs])
                        hsk.append(t)
                for m in range(4):
                    ps = psum.tile([128, FC], F32, tag="mm", name="mm")
                    for k in range(2):
                        nc.tensor.matmul(ps[:], lhsT=w_in_x_sb[k][:, m * 128:(m + 1) * 128],
                                         rhs=hsk[k][:], start=(k == 0), stop=(k == 1))
                    nc.scalar.activation(u[m][:, fs], ps[:], AF.Silu)
                zjobs.append((fs, hsk))
                ps = psum.tile([128, FC], F32, tag="mm", name="mm")
                for k in range(4):
                    nc.tensor.matmul(ps[0:64, :], lhsT=w_x_sb[k][:], rhs=u[k][:, fs],
                                     start=(k == 0), stop=(k == 3))
                nc.scalar.copy(xd16[:, fs], ps[0:16, :])
                nc.scalar.copy(bc16[:, fs], ps[32:64, :])
                nc.sync.dma_start(out=bc_dram[0, 0:32:2, gs], in_=bc16[0:16, fs])
                nc.sync.dma_start(out=bc_dram[0, 1:32:2, gs], in_=bc16[16:32, fs])

            delta = [big.tile([128, LCMAX], F16, tag=f"delta{m}", name=f"delta{m}") for m in range(2)]
            du = big.tile([128, 2, LCMAX], BF16, tag="dup", name="dup")
            sps = []
            for m in range(2):
                for fc in range(nf):
                    fs = slice(fc * FC, (fc + 1) * FC)
                    ps = psum.tile([128, FC], F32, tag="mm", name="mm")
                    nc.tensor.matmul(ps[:], lhsT=w_dt_sb[:, m * 128:(m + 1) * 128],
                                     rhs=xd16[:, fs], start=True, stop=True)
                    # softplus(x) = ln(exp(x) + 1); x = raw + bdt stays < ~3 here
                    sp = work.tile([128, FC], F32, tag="sp", name="sp", bufs=4)
                    nc.scalar.activation(sp[:], ps[:], AF.Exp, bias=bdt_sb[m][:])
                    sps.append((m, fc, sp))
            for m, fc, sp in sps:
                fs = slice(fc * FC, (fc + 1) * FC)
                nc.scalar.activation(delta[m][:, fs], sp[:], AF.Ln, bias=1.0)
            for m in range(2):
                nc.vector.tensor_tensor(du[:, m, :size], delta[m][:, :size],
                                        u[m][:, :size], OP.mult)
            return u, z16, delta, du, zjobs

        def emit_z(z16, zjobs):
            """z-projection + silu (gate input), emitted after the scan stage so
            the silus never preempt the delta->dA critical path on ACT."""
            for fs, hsk in zjobs:
                for m in range(2):
                    ps = psum.tile([128, FC], F32, tag="mm", name="mm")
                    for k in range(2):
                        nc.tensor.matmul(ps[:], lhsT=w_in_z_sb[k][:, m * 128:(m + 1) * 128],
                                         rhs=hsk[k][:], start=(k == 0), stop=(k == 1))
                    nc.scalar.activation(z16[m][:, fs], ps[:], AF.Silu)

        def scanstage(cid, base, size, u, delta, du, mid_emit=None):
            """Scan chunk cid; returns (yps, next-pre). mid_emit is invoked
            after state MID_N so next-chunk projections interleave."""
            ls = slice(base, base + size)
            nq = size // 512
            yps = [[psumy.tile([128, 512], F32, tag=f"yps{m}_{q}", name=f"yps{m}_{q}")
                    for q in range(nq)] for m in range(2)]
            # D-term: yps = diag(D) @ u  (starts the accumulation group)
            for m in range(2):
                for q in range(nq):
                    qs = slice(q * 512, (q + 1) * 512)
                    nc.tensor.matmul(yps[m][q][:], lhsT=d_diag_sb[m][:], rhs=u[m][:, qs],
                                     start=True, stop=False, skip_group_check=True)
            nxt = None
            for n in range(N_STATE):
                if n == MID_N and mid_emit is not None:
                    nxt = mid_emit()
                bbcb = work.tile([128, 2, LCMAX], BF16, tag="bbcb", name="bbcb", bufs=6)
                nc.sync.dma_start(out=bbcb[:, :, :size],
                                  in_=bc_dram[0:1, 2 * n:2 * n + 2, ls].to_broadcast([128, 2, size]))
                bb = bbcb[:, 0:1, :size]
                cb = bbcb[:, 1:2, :size]
                # paired across m: one wide TT for both halves (shared B/C row
                # via a zero-stride broadcast dim) halves the per-op overhead
                dbu = work.tile([128, 2, LCMAX], BF16, tag="dbu", name="dbu", bufs=5)
                dbu_eng = nc.gpsimd if n in POOL_DBU else nc.vector
                dbu_eng.tensor_tensor(dbu[:, :, :size], du[:, :, :size],
                                      bb.to_broadcast([128, 2, size]), OP.mult)
                h = work.tile([128, 2, LCMAX], BF16, tag="h", name="h", bufs=4)
                t = work.tile([128, 2, LCMAX], BF16, tag="t", name="t", bufs=4)
                for m in range(2):
                    da = work.tile([128, LCMAX], F16, tag="da", name="da", bufs=32)
                    nc.scalar.activation(da[:, :size], delta[m][:, :size], AF.Exp,
                                         scale=a_sc_sb[m][:, n:n + 1])
                    init = 0.0 if cid == 0 else hlast[m][n][:]
                    nc.vector.tensor_tensor_scan(h[:, m, :size], da[:, :size],
                                                 dbu[:, m, :size], init,
                                                 OP.mult, OP.add)
                    if cid + 1 < len(CHUNKS):
                        nc.gpsimd.tensor_copy(hlast[m][n][:], h[:, m, size - 1:size])
                t_eng = nc.gpsimd if n in POOL_T else nc.vector
                t_eng.tensor_tensor(t[:, :, :size], h[:, :, :size],
                                    cb.to_broadcast([128, 2, size]), OP.mult)
                for m in range(2):
                    for q in range(size // 512):
                        qs = slice(q * 512, (q + 1) * 512)
                        nc.tensor.matmul(yps[m][q][:], lhsT=ident_sb[:], rhs=t[:, m, qs],
                                         start=False, stop=(n == N_STATE - 1),
                                         skip_group_check=True)
            return yps, nxt

        def poststage(cid, base, size, z16, yps):
            """Gate + out_proj partial for chunk cid."""
            y = [work.tile([128, LCMAX], BF16, tag=f"y{m}", name=f"y{m}", bufs=2) for m in range(2)]
            for m in range(2):
                for q in range(size // 512):
                    qs = slice(q * 512, (q + 1) * 512)
                    nc.vector.tensor_tensor(y[m][:, qs], yps[m][q][:], z16[m][:, qs], OP.mult)
            for oi in range(2):
                for fc in range(size // FC):
                    fs = slice(fc * FC, (fc + 1) * FC)
                    gs = slice(base + fc * FC, base + (fc + 1) * FC)
                    ps = psum.tile([128, FC], F32, tag="mm", name="mm")
                    for k in range(2):
                        nc.tensor.matmul(ps[:], lhsT=w_out_sb[k][:, oi * 128:(oi + 1) * 128],
                                         rhs=y[k][:, fs], start=(k == 0), stop=(k == 1))
                    ob = work.tile([128, FC], BF16, tag="ob", name="ob")
                    nc.scalar.copy(ob[:], ps[:])
                    nc.sync.dma_start(out=out_ap[oi][:, gs], in_=ob[:])

        # ---- software pipeline over L-chunks ----
        bases = [sum(CHUNKS[:i]) for i in range(len(CHUNKS))]
        nchunks = len(CHUNKS)
        pre = prestage(0, bases[0], CHUNKS[0])
        for cid, (base, size) in enumerate(zip(bases, CHUNKS)):
            yps, _ = scanstage(cid, base, size, pre[0], pre[2], pre[3], None)
            emit_z(pre[1], pre[4])
            nxt = None
            if cid + 1 < nchunks:
                nxt = prestage(cid + 1, bases[cid + 1], CHUNKS[cid + 1])
            poststage(cid, base, size, pre[1], yps)
            pre = nxt

    nc.compile()
    _CACHE["nc"] = nc
    return nc


def _in_maps(inputs):
    import ml_dtypes
    BF = ml_dtypes.bfloat16
    f = lambda a: np.ascontiguousarray(np.asarray(a), dtype=np.float32)
    g = lambda a: np.ascontiguousarray(np.asarray(a, dtype=np.float32), dtype=BF)
    hs = f(inputs["hidden_states"])          # [2, L, 256]
    W_in = f(inputs["W_in"])                 # [2048, 256]
    W_out = f(inputs["W_out"])               # [256, 1024]
    maps = []
    for branch in range(2):
        sfx = "f" if branch == 0 else "b"
        Wx0 = f(inputs[f"Wx_{sfx}"])         # [48, 512]
        Wx = np.zeros((64, 512), np.float32)  # dt rows 0:16, B 32:48, C 48:64
        Wx[0:16] = Wx0[0:16]
        Wx[32:48] = Wx0[16:32]
        Wx[48:64] = Wx0[32:48]
        Wdt = f(inputs[f"Wdt_{sfx}"])        # [512, 16]
        bdt = f(inputs[f"bdt_{sfx}"])        # [512]
        A = -np.exp(f(inputs[f"A_log_{sfx}"]))   # [512, 16]
        D = f(inputs[f"D_{sfx}"])            # [512]
        xrows = W_in[branch * 1024: branch * 1024 + 512]
        zrows = W_in[branch * 1024 + 512: branch * 1024 + 1024]
        for b in range(2):
            hsT = hs[b].T                    # [256, L]
            if branch == 1:
                hsT = hsT[:, ::-1]
            for half in range(2):
                mine = np.arange(256 * half, 256 * half + 256)
                perm = np.r_[mine, np.arange(256 * (1 - half), 256 * (1 - half) + 256)]
                ddiag = np.zeros((2, 128, 128), np.float32)
                for m in range(2):
                    np.fill_diagonal(ddiag[m], D[mine][m * 128:(m + 1) * 128])
                wdt_block = np.zeros((128, 256), np.float32)
                wdt_block[0:16] = Wdt[mine].T
                wout_blk = W_out[:, branch * 512 + 256 * half:
                                 branch * 512 + 256 * half + 256].T.reshape(2, 128, 256)
                wpackA = xrows[perm].T.reshape(2, 128, 512).transpose(1, 0, 2).reshape(128, 1024)
                wpackB = np.concatenate([
                    zrows[mine].T.reshape(2, 128, 256).transpose(1, 0, 2).reshape(128, 512),
                    Wx[:, perm].T.reshape(4, 128, 64).transpose(1, 0, 2).reshape(128, 256),
                    wdt_block,
                    ddiag.transpose(1, 0, 2).reshape(128, 256),
                    wout_blk.transpose(1, 0, 2).reshape(128, 512),
                    np.eye(128, dtype=np.float32),
                ], axis=1)
                fpack = np.ascontiguousarray(np.concatenate([
                    bdt[mine].reshape(2, 128, 1).transpose(1, 0, 2).reshape(128, 2),
                    A[mine].reshape(2, 128, 16).transpose(1, 0, 2).reshape(128, 32),
                ], axis=1), dtype=np.float32)
                m = {
                    "hsT": g(hsT).reshape(2, 128, L),
                    "wpackA": g(wpackA),
                    "wpackB": np.concatenate([g(wpackB), fpack.view(BF)], axis=1),
                }
                maps.append(m)
    # maps order: branch-major, then b, then half -> core = (branch*2+b)*2+half
    return maps


def _run(inputs, trace=False):
    nc = _build()
    maps = _in_maps(inputs)
    res = run_bass_kernel_spmd(nc, maps, core_ids=list(range(8)), trace=trace)
    outs = [r["out"].astype(np.float32).reshape(256, L) for r in res.results]
    out = np.empty((2, L, D_MODEL), np.float32)
    for b in range(2):
        fwd = outs[2 * b] + outs[2 * b + 1]
        bwd = outs[4 + 2 * b] + outs[4 + 2 * b + 1]
        out[b] = (fwd + bwd[:, ::-1]).T
    return out, res


def kernel(**inputs):
    out, _ = _run(inputs, trace=False)
    return out

